# revision 34
# baseline (speedup 1.0000x reference)
"""Trainium2 Bass kernel for nn_CrossModalAttention (B=32768, D=1024, H=16, OUT=3).

Math notes (exact simplifications of the reference):
  - Attention is over a single key (seq len 1) -> softmax == 1.0 exactly, so the
    attention output is just v @ wo.T + bo with v = xkv @ wv.T + bv.
  - (xkv @ wv.T + bv) @ wo.T + bo == xkv @ (wo @ wv).T + (wo @ bv + bo): each
    block collapses to ONE [B,D]x[D,D] matmul (weights fused on host in fp64).
  - ln_g/ln_b are folded into the fc1 weights/bias (exact algebra), so the
    layernorm on device is a pure normalize: y = (u - mean) * rsqrt(var + eps).
  - fc1 of the concat [ta|tv|av] splits into 3 per-block matmuls accumulated
    in PSUM.

Perf notes (v2, measured on this axon-tunneled trn2):
  - bf16 matmuls stream at exactly 1 cyc/row (437us for 1.05M rows); float32r
    measured 1.7x slower (exposed stationary loads) -> all GEMMs in bf16,
    fp32 PSUM accumulation.
  - DMA bandwidth measured ~160-190 GB/s/core (not the 332 spec'd), so all
    weights (12 MB in bf16) stay RESIDENT in SBUF; only activations stream
    (24 MB bf16 per core) on two hardware queues (SP + Act).
  - LayerNorm stats: per-j-tile partial sums/squares tree-accumulated on the
    otherwise-idle Pool engine; one 512-row PE matmul per stat reduces
    partitions.  Stats/scale path is kept fp32; fc2 weights are split-bf16
    (hi+lo) so their rounding cancels.

Data parallel over 8 cores: batch 32768 -> 8 x 4096, weights replicated.
"""

import sys

sys.path.insert(0, "/opt/trn_rl_repo")

import numpy as np

import concourse.bass as bass
import concourse.mybir as mybir
import concourse.tile as tile
from concourse import bacc
from concourse.bass_utils import run_bass_kernel_spmd

F32 = mybir.dt.float32
F32R = mybir.dt.float32r
BF16 = mybir.dt.bfloat16
ADD = mybir.AluOpType.add
SUB = mybir.AluOpType.subtract
MUL = mybir.AluOpType.mult
ACT_F = mybir.ActivationFunctionType

B, D, OUT = 32768, 1024, 3
NCORES = 8
LN_EPS = 1e-5
JT = D // 128  # 8 feature tiles
KT = D // 128  # 8 contraction tiles

_cache: dict = {}


def _build(b_loc: int, bc: int, reps: int = 1):
    """Build + compile the per-core Bass module (SPMD, same on all cores).

    reps>1 repeats the whole chunk loop inside the NEFF (same data) — used
    only for timing: (t[reps=3]-t[reps=1])/2 cancels dispatch overhead."""
    nch = b_loc // bc
    nc = bacc.Bacc(None, target_bir_lowering=False, debug=False)

    # ---- DRAM I/O ----
    # activations, chunk-major so each chunk DMA is 128 descriptors x 8KB
    xt_d = nc.dram_tensor("xt", [nch, 128, KT, bc], BF16, kind="ExternalInput")
    xa_d = nc.dram_tensor("xa", [nch, 128, KT, bc], BF16, kind="ExternalInput")
    xv_d = nc.dram_tensor("xv", [nch, 128, KT, bc], BF16, kind="ExternalInput")
    # fused attention weights [i, p, j, k, jcol]; fc1 (ln-folded) same layout
    wtt_d = nc.dram_tensor("wtt", [3, 128, JT, KT, 128], BF16, kind="ExternalInput")
    gtt_d = nc.dram_tensor("gtt", [3, 128, JT, KT, 128], BF16, kind="ExternalInput")
    bsw_d = nc.dram_tensor("bsw", [3, 128, JT], F32, kind="ExternalInput")
    f1b_d = nc.dram_tensor("f1b", [128, JT], F32, kind="ExternalInput")
    f2hl_d = nc.dram_tensor("f2hl", [128, KT, 2 * OUT], BF16, kind="ExternalInput")
    onec_d = nc.dram_tensor("onec", [128, 1], BF16, kind="ExternalInput")
    oner_d = nc.dram_tensor("oner", [1, 128], BF16, kind="ExternalInput")
    out_d = nc.dram_tensor("outT", [nch, 2 * OUT, bc], F32, kind="ExternalOutput")

    blocks = [("t", "a"), ("t", "v"), ("a", "v")]  # (query/residual, key-value)

    with tile.TileContext(nc) as tc:
        with (
            tc.tile_pool(name="const", bufs=1) as const,
            tc.tile_pool(name="wres", bufs=1) as wres,
            tc.tile_pool(name="xp", bufs=2) as xp,
            tc.tile_pool(name="up", bufs=3) as up,
            tc.tile_pool(name="sqp", bufs=2) as sqp,
            tc.tile_pool(name="ptp", bufs=1) as ptp,
            tc.tile_pool(name="srow", bufs=1) as srow,
            tc.tile_pool(name="bcp", bufs=1) as bcp,
            tc.tile_pool(name="hp", bufs=1) as hp,
            tc.tile_pool(name="op", bufs=1) as op_pool,
            tc.tile_pool(name="mmps", bufs=4, space="PSUM") as mmps,
            tc.tile_pool(name="stps", bufs=1, space="PSUM") as stps,
            tc.tile_pool(name="bcps", bufs=1, space="PSUM") as bcps,
            tc.tile_pool(name="ops", bufs=1, space="PSUM") as ops,
        ):
            # ---- resident constants / weights ----
            ones_col = const.tile([128, 1], BF16, tag="ones_col")
            nc.sync.dma_start(ones_col, onec_d[:, :])
            ones_row = const.tile([1, 128], BF16, tag="ones_row")
            nc.sync.dma_start(ones_row, oner_d[:, :])
            eps_sb = const.tile([1, 1], F32, tag="eps")
            nc.vector.memset(eps_sb, LN_EPS)
            b_sb = []
            for i in range(3):
                t = const.tile([128, JT], F32, tag=f"b{i}")
                nc.sync.dma_start(t, bsw_d[i, :, :])
                b_sb.append(t)
            f1b_sb = const.tile([128, JT], F32, tag="f1b")
            nc.sync.dma_start(f1b_sb, f1b_d[:, :])

            # Launch-head DMAs, two HW queues (SP + Act), ordered by first use:
            # attn weights + chunk-0 activations first, fc1 weights behind.
            def xtile(mod, dram, c, eng):
                t = xp.tile([128, KT, bc], BF16, tag=f"x{mod}")
                eng.dma_start(t, dram[c, :, :, :])
                return t

            # weight tiles split in j-halves so chunk-0 compute can start as
            # soon as the first half lands; head DMAs spread over THREE
            # queues (SP + Act HWDGE, Pool SWDGE) ordered by first use
            JH = JT // 2
            w_sb = [
                [
                    wres.tile([128, JH, KT, 128], BF16, tag=f"w{i}{h}", name=f"w{i}{h}")
                    for h in range(2)
                ]
                for i in range(3)
            ]
            g_sb = [
                [
                    wres.tile([128, JH, KT, 128], BF16, tag=f"g{i}{h}", name=f"g{i}{h}")
                    for h in range(2)
                ]
                for i in range(3)
            ]
            x0 = {}
            x0["a"] = xtile("a", xa_d, 0, nc.scalar)
            nc.sync.dma_start(w_sb[0][0], wtt_d[0, :, 0:JH, :, :])
            nc.scalar.dma_start(w_sb[0][1], wtt_d[0, :, JH:JT, :, :])
            x0["t"] = xtile("t", xt_d, 0, nc.scalar)
            x0["v"] = xtile("v", xv_d, 0, nc.sync)
            nc.sync.dma_start(w_sb[1][0], wtt_d[1, :, 0:JH, :, :])
            nc.scalar.dma_start(w_sb[1][1], wtt_d[1, :, JH:JT, :, :])
            nc.sync.dma_start(w_sb[2][0], wtt_d[2, :, 0:JH, :, :])
            nc.scalar.dma_start(w_sb[2][1], wtt_d[2, :, JH:JT, :, :])
            nc.sync.dma_start(g_sb[0][0], gtt_d[0, :, 0:JH, :, :])
            nc.scalar.dma_start(g_sb[1][0], gtt_d[1, :, 0:JH, :, :])
            nc.sync.dma_start(g_sb[2][0], gtt_d[2, :, 0:JH, :, :])
            nc.scalar.dma_start(g_sb[0][1], gtt_d[0, :, JH:JT, :, :])
            nc.sync.dma_start(g_sb[1][1], gtt_d[1, :, JH:JT, :, :])
            nc.scalar.dma_start(g_sb[2][1], gtt_d[2, :, JH:JT, :, :])
            f2hl_sb = const.tile([128, KT, 2 * OUT], BF16, tag="f2hl")
            nc.sync.dma_start(f2hl_sb, f2hl_d[:, :, :])

            for rep in range(reps):
              for c in range(nch):
                if c == 0 and rep == 0:
                    x_sb = x0
                else:
                    # all on the idle SP queue: issuing from Act would delay
                    # the transfer behind the previous chunk's Act compute
                    x_sb = {
                        "a": xtile("a", xa_d, c, nc.sync),
                        "t": xtile("t", xt_d, c, nc.sync),
                        "v": xtile("v", xv_d, c, nc.sync),
                    }

                u_sb = [None, None, None]
                stat = [None, None, None]  # (pu, py) f32 partials
                rowv = [None, None, None]  # (m_sb, r_sb)
                bcst = [None, None, None]  # (mbc, rbc) f32 sbuf

                def u_mm(i, jlo, jhi):
                    """attention matmuls for j in [jlo,jhi); evac+square+tree
                    ride along on DVE/Act (all-bf16 SBUF -> DVE 4x mode)."""
                    qm, km = blocks[i]
                    xq, xkv = x_sb[qm], x_sb[km]
                    if jlo == 0:
                        u_sb[i] = up.tile([128, JT, bc], BF16, tag="u", name="u")
                        stat[i] = (
                            ptp.tile([128, bc], BF16, tag="pu", name="pu"),
                            ptp.tile([128, bc], BF16, tag="py", name="py"),
                        )
                    u, (pu, py) = u_sb[i], stat[i]
                    for j in range(jlo, jhi):
                        u_ps = mmps.tile([128, bc], F32, tag="mm")
                        wt = w_sb[i][j // 4]
                        for k in range(KT):
                            nc.tensor.matmul(
                                u_ps, wt[:, j % 4, k, :], xkv[:, k, :],
                                start=(k == 0), stop=(k == KT - 1),
                            )
                        # evacuate PSUM + bias + residual in one DVE pass
                        nc.vector.scalar_tensor_tensor(
                            out=u[:, j, :], in0=u_ps,
                            scalar=b_sb[i][:, j : j + 1], in1=xq[:, j, :],
                            op0=ADD, op1=ADD,
                        )
                        sq = sqp.tile([128, bc], BF16, tag="sq")
                        nc.scalar.activation(sq, u[:, j, :], ACT_F.Square)
                        if j == 0:
                            nc.vector.tensor_copy(pu, u[:, j, :])
                            nc.vector.tensor_copy(py, sq)
                        else:
                            nc.vector.tensor_add(pu, pu, u[:, j, :])
                            nc.vector.tensor_add(py, py, sq)

                def s_mm(i):
                    """partition-reduce partials -> s1/s2, then small-op chain
                    to mean + inv-std rows (f32)."""
                    pu, py = stat[i]
                    # s1/s2 share one PSUM bank (base partitions 0 and 32)
                    st = stps.tile([64, bc], F32, tag="s12")
                    s1 = st[0:1, :]
                    s2 = st[32:33, :]
                    nc.tensor.matmul(s1, ones_col, pu, start=True, stop=True)
                    nc.tensor.matmul(s2, ones_col, py, start=True, stop=True)
                    # m/r rows in bf16: the broadcast copies round to bf16
                    # anyway, and bf16 stationaries avoid the slow f32r
                    # weight-load path in the broadcast matmuls
                    m_sb = srow.tile([1, bc], BF16, tag="m")
                    nc.scalar.mul(m_sb, s1, 1.0 / D)
                    msq = srow.tile([1, bc], F32, tag="msq")
                    nc.vector.tensor_mul(msq, m_sb, m_sb)
                    var = srow.tile([1, bc], F32, tag="var")
                    nc.vector.scalar_tensor_tensor(
                        out=var, in0=s2, scalar=1.0 / D, in1=msq, op0=MUL, op1=SUB
                    )
                    std = srow.tile([1, bc], F32, tag="std")
                    nc.scalar.activation(std, var, ACT_F.Sqrt, bias=eps_sb, scale=1.0)
                    r_sb = srow.tile([1, bc], BF16, tag="r")
                    with nc.allow_low_precision(reason="bf16 LN scale by design"):
                        nc.vector.reciprocal(r_sb, std)
                    rowv[i] = (m_sb, r_sb)

                def bc_mm(i):
                    """broadcast mean + inv-std across partitions (K=1 matmul),
                    copy to SBUF f32, then normalize u in place -> bf16 y."""
                    m_sb, r_sb = rowv[i]
                    mbc_ps = bcps.tile([128, bc], F32, tag="mbc")
                    nc.tensor.matmul(mbc_ps, ones_row, m_sb, start=True, stop=True)
                    rbc_ps = bcps.tile([128, bc], F32, tag="rbc")
                    nc.tensor.matmul(rbc_ps, ones_row, r_sb, start=True, stop=True)
                    mbc = bcp.tile([128, bc], BF16, tag="mbc_sb")
                    nc.vector.tensor_copy(mbc, mbc_ps)
                    rbc = bcp.tile([128, bc], BF16, tag="rbc_sb")
                    nc.vector.tensor_copy(rbc, rbc_ps)
                    bcst[i] = (mbc, rbc)

                def norm(i):
                    u = u_sb[i]
                    mbc, rbc = bcst[i]
                    for j in range(JT):
                        nc.vector.tensor_sub(u[:, j, :], u[:, j, :], mbc)
                        nc.vector.tensor_mul(u[:, j, :], u[:, j, :], rbc)

                def fc1(j2, ilo, ihi, z_ps):
                    for i in range(ilo, ihi):
                        gt = g_sb[i][j2 // 4]
                        for k in range(KT):
                            nc.tensor.matmul(
                                z_ps, gt[:, j2 % 4, k, :], u_sb[i][:, k, :],
                                start=(i == 0 and k == 0),
                                stop=(i == 2 and k == KT - 1),
                            )

                # ---- PE emission order: keep the PE streaming while the
                # stats latency chains (DVE/Act/Pool) resolve in parallel ----
                def relu_evac(j2, z_ps):
                    # on DVE (not Act): h = max(z + b, 0); keeps the fc1 PSUM
                    # recycle path off the Act queue (busy with squares)
                    nc.vector.tensor_scalar(
                        out=h_sb[:, j2, :], in0=z_ps,
                        scalar1=f1b_sb[:, j2 : j2 + 1], scalar2=0.0,
                        op0=ADD, op1=mybir.AluOpType.max,
                    )

                u_mm(0, 0, JT)
                u_mm(1, 0, 2)
                s_mm(0)
                u_mm(1, 2, JT)
                bc_mm(0)
                u_mm(2, 0, 2)
                s_mm(1)
                u_mm(2, 2, JT)
                norm(0)
                bc_mm(1)

                h_sb = hp.tile([128, JT, bc], BF16, tag="h")
                z01 = [
                    mmps.tile([128, bc], F32, tag="mm", name=f"z{t_}")
                    for t_ in range(2)
                ]
                fc1(0, 0, 1, z01[0])
                s_mm(2)
                norm(1)
                fc1(1, 0, 1, z01[1])
                fc1(0, 1, 2, z01[0])
                bc_mm(2)
                norm(2)
                fc1(1, 1, 2, z01[1])
                fc1(0, 2, 3, z01[0])
                relu_evac(0, z01[0])
                fc1(1, 2, 3, z01[1])
                relu_evac(1, z01[1])
                for j2 in range(2, JT):
                    z_ps = mmps.tile([128, bc], F32, tag="mm")
                    fc1(j2, 0, 3, z_ps)
                    relu_evac(j2, z_ps)

                # fc2 hi+lo split-bf16 folded into ONE 8-matmul pass: lhsT
                # [128, 6] = [hi | lo] per k-tile, output rows 0-2 + 3-5 both
                # hold partial sums; the final Act evac adds them. Full fp32
                # fc2-weight precision at single-pass PE cost.
                o_ps = ops.tile([2 * OUT, bc], F32, tag="o")
                for k in range(KT):
                    nc.tensor.matmul(
                        o_ps, f2hl_sb[:, k, :], h_sb[:, k, :],
                        start=(k == 0), stop=(k == KT - 1),
                    )
                o_sb = op_pool.tile([2 * OUT, bc], F32, tag="osb")
                nc.scalar.activation(o_sb, o_ps, ACT_F.Identity)
                nc.scalar.dma_start(out_d[c, :, :], o_sb)

    nc.compile()
    return nc


def _swizzle_weight(wt: np.ndarray, bf16) -> np.ndarray:
    """[D_in, D_out] contraction-major matrix -> [128, JT, KT, 128] where
    tile[p, j, k, jc] = wt[k*128+p, j*128+jc]."""
    kt, jt = wt.shape[0] // 128, wt.shape[1] // 128
    return np.ascontiguousarray(
        wt.reshape(kt, 128, jt, 128).transpose(1, 2, 0, 3).astype(bf16)
    )


def _swizzle_x(shard: np.ndarray, bc: int, bf16) -> np.ndarray:
    """[b_loc, D] activation shard -> chunked feature-major [nch, 128, KT, bc]."""
    b_loc = shard.shape[0]
    nch = b_loc // bc
    return np.ascontiguousarray(
        shard.astype(bf16).reshape(nch, bc, KT, 128).transpose(0, 3, 2, 1)
    )


def _prep_shared(w_qkv, b_qkv, w_o, b_o, ln_g, ln_b, fc1_w, fc1_b, fc2_w, fc2_b):
    import ml_dtypes

    bf16 = ml_dtypes.bfloat16
    f6 = np.float64
    wtt = np.empty((3, 128, JT, KT, 128), bf16)
    bsw = np.empty((3, 128, JT), np.float32)
    gtt = np.empty((3, 128, JT, KT, 128), bf16)
    f1b_full = fc1_b.astype(f6).copy()
    for i in range(3):
        wv, bv = w_qkv[i, 2].astype(f6), b_qkv[i, 2].astype(f6)
        wo, bo = w_o[i].astype(f6), b_o[i].astype(f6)
        w_i = wo @ wv                      # [j_out, d_in]
        bias_i = wo @ bv + bo              # [j_out]
        wtt[i] = _swizzle_weight(np.ascontiguousarray(w_i.T).astype(np.float32), bf16)
        bsw[i] = bias_i.astype(np.float32).reshape(JT, 128).T
        f_i = fc1_w[:, i * D : (i + 1) * D].astype(f6)   # [h, j]
        g_i = f_i * ln_g[i].astype(f6)[None, :]
        f1b_full += f_i @ ln_b[i].astype(f6)
        gtt[i] = _swizzle_weight(np.ascontiguousarray(g_i.T).astype(np.float32), bf16)
    f1bsw = f1b_full.astype(np.float32).reshape(JT, 128).T.copy()
    f2t = np.ascontiguousarray(fc2_w.astype(np.float32).T)     # [D, OUT]
    f2hi = f2t.astype(bf16)
    f2lo = (f2t - f2hi.astype(np.float32)).astype(bf16)
    f2hsw = f2hi.reshape(KT, 128, OUT).transpose(1, 0, 2)
    f2lsw = f2lo.reshape(KT, 128, OUT).transpose(1, 0, 2)
    # hi|lo concatenated into one [128, KT, 6] stationary: the 6-row matmul
    # output carries both partials; they are summed on the host
    f2hl = np.ascontiguousarray(np.concatenate([f2hsw, f2lsw], axis=2))
    return dict(
        wtt=wtt, gtt=gtt, bsw=bsw, f1b=np.ascontiguousarray(f1bsw),
        f2hl=f2hl,
        onec=np.ones((128, 1), bf16), oner=np.ones((1, 128), bf16),
    )


def _make_in_maps(
    text_x, audio_x, video_x, w_qkv, b_qkv, w_o, b_o, ln_g, ln_b,
    fc1_w, fc1_b, fc2_w, fc2_b, bc,
):
    import ml_dtypes

    bf16 = ml_dtypes.bfloat16
    b_total = text_x.shape[0]
    b_loc = b_total // NCORES
    shared = _prep_shared(
        np.asarray(w_qkv), np.asarray(b_qkv), np.asarray(w_o), np.asarray(b_o),
        np.asarray(ln_g), np.asarray(ln_b), np.asarray(fc1_w),
        np.asarray(fc1_b), np.asarray(fc2_w), np.asarray(fc2_b),
    )
    in_maps = []
    for cidx in range(NCORES):
        sl = slice(cidx * b_loc, (cidx + 1) * b_loc)
        in_maps.append(
            dict(
                xt=_swizzle_x(text_x[sl], bc, bf16),
                xa=_swizzle_x(audio_x[sl], bc, bf16),
                xv=_swizzle_x(video_x[sl], bc, bf16),
                **shared,
            )
        )
    return in_maps


def kernel(
    text_x, audio_x, video_x, w_qkv, b_qkv, w_o, b_o, ln_g, ln_b,
    fc1_w, fc1_b, fc2_w, fc2_b, num_heads=16,
):
    text_x = np.asarray(text_x, np.float32)
    audio_x = np.asarray(audio_x, np.float32)
    video_x = np.asarray(video_x, np.float32)
    b_total = text_x.shape[0]
    b_loc = b_total // NCORES
    bc = min(512, b_loc)

    key = (b_loc, bc)
    if key not in _cache:
        _cache[key] = _build(b_loc, bc)
    nc = _cache[key]

    in_maps = _make_in_maps(
        text_x, audio_x, video_x, w_qkv, b_qkv, w_o, b_o, ln_g, ln_b,
        fc1_w, fc1_b, fc2_w, fc2_b, bc,
    )
    res = run_bass_kernel_spmd(nc, in_maps, core_ids=list(range(NCORES)))
    out = np.empty((b_total, OUT), np.float32)
    f2bv = np.asarray(fc2_b, np.float32).reshape(1, OUT)
    for cidx in range(NCORES):
        o = res.results[cidx]["outT"]  # [nch, 2*OUT, bc]: rows 0-2 hi, 3-5 lo
        o = o[:, :OUT, :] + o[:, OUT:, :]
        out[cidx * b_loc : (cidx + 1) * b_loc] = (
            o.transpose(0, 2, 1).reshape(b_loc, OUT) + f2bv
        )
    return out


# revision 35
# speedup vs baseline: 1.0220x; 1.0220x over previous
"""Trainium2 Bass kernel for nn_CrossModalAttention (B=32768, D=1024, H=16, OUT=3).

Math notes (exact simplifications of the reference):
  - Attention is over a single key (seq len 1) -> softmax == 1.0 exactly, so the
    attention output is just v @ wo.T + bo with v = xkv @ wv.T + bv.
  - (xkv @ wv.T + bv) @ wo.T + bo == xkv @ (wo @ wv).T + (wo @ bv + bo): each
    block collapses to ONE [B,D]x[D,D] matmul (weights fused on host in fp64).
  - ln_g/ln_b are folded into the fc1 weights/bias (exact algebra), so the
    layernorm on device is a pure normalize: y = (u - mean) * rsqrt(var + eps).
  - fc1 of the concat [ta|tv|av] splits into 3 per-block matmuls accumulated
    in PSUM.

Perf notes (v2, measured on this axon-tunneled trn2):
  - bf16 matmuls stream at exactly 1 cyc/row (437us for 1.05M rows); float32r
    measured 1.7x slower (exposed stationary loads) -> all GEMMs in bf16,
    fp32 PSUM accumulation.
  - DMA bandwidth measured ~160-190 GB/s/core (not the 332 spec'd), so all
    weights (12 MB in bf16) stay RESIDENT in SBUF; only activations stream
    (24 MB bf16 per core) on two hardware queues (SP + Act).
  - LayerNorm stats: per-j-tile partial sums/squares tree-accumulated on the
    otherwise-idle Pool engine; one 512-row PE matmul per stat reduces
    partitions.  Stats/scale path is kept fp32; fc2 weights are split-bf16
    (hi+lo) so their rounding cancels.

Data parallel over 8 cores: batch 32768 -> 8 x 4096, weights replicated.
"""

import sys

sys.path.insert(0, "/opt/trn_rl_repo")

import numpy as np

import concourse.bass as bass
import concourse.mybir as mybir
import concourse.tile as tile
from concourse import bacc
from concourse.bass_utils import run_bass_kernel_spmd

F32 = mybir.dt.float32
F32R = mybir.dt.float32r
BF16 = mybir.dt.bfloat16
ADD = mybir.AluOpType.add
SUB = mybir.AluOpType.subtract
MUL = mybir.AluOpType.mult
ACT_F = mybir.ActivationFunctionType

B, D, OUT = 32768, 1024, 3
NCORES = 8
LN_EPS = 1e-5
JT = D // 128  # 8 feature tiles
KT = D // 128  # 8 contraction tiles

_cache: dict = {}


def _build(b_loc: int, bc: int, reps: int = 1):
    """Build + compile the per-core Bass module (SPMD, same on all cores).

    reps>1 repeats the whole chunk loop inside the NEFF (same data) — used
    only for timing: (t[reps=3]-t[reps=1])/2 cancels dispatch overhead."""
    nch = b_loc // bc
    nc = bacc.Bacc(None, target_bir_lowering=False, debug=False)

    # ---- DRAM I/O ----
    # activations, chunk-major so each chunk DMA is 128 descriptors x 8KB
    xt_d = nc.dram_tensor("xt", [nch, 128, KT, bc], BF16, kind="ExternalInput")
    xa_d = nc.dram_tensor("xa", [nch, 128, KT, bc], BF16, kind="ExternalInput")
    xv_d = nc.dram_tensor("xv", [nch, 128, KT, bc], BF16, kind="ExternalInput")
    # fused attention weights [i, p, j, k, jcol]; fc1 (ln-folded) same layout
    wtt_d = nc.dram_tensor("wtt", [3, 128, JT, KT, 128], BF16, kind="ExternalInput")
    gtt_d = nc.dram_tensor("gtt", [3, 128, JT, KT, 128], BF16, kind="ExternalInput")
    bsw_d = nc.dram_tensor("bsw", [3, 128, JT], F32, kind="ExternalInput")
    f1b_d = nc.dram_tensor("f1b", [128, JT], F32, kind="ExternalInput")
    f2hl_d = nc.dram_tensor("f2hl", [128, KT, 2 * OUT], BF16, kind="ExternalInput")
    onec_d = nc.dram_tensor("onec", [128, 1], BF16, kind="ExternalInput")
    oner_d = nc.dram_tensor("oner", [1, 128], F32R, kind="ExternalInput")
    out_d = nc.dram_tensor("outT", [nch, 2 * OUT, bc], F32, kind="ExternalOutput")

    blocks = [("t", "a"), ("t", "v"), ("a", "v")]  # (query/residual, key-value)

    with tile.TileContext(nc) as tc:
        with (
            tc.tile_pool(name="const", bufs=1) as const,
            tc.tile_pool(name="wres", bufs=1) as wres,
            tc.tile_pool(name="xp", bufs=2) as xp,
            tc.tile_pool(name="up", bufs=3) as up,
            tc.tile_pool(name="sqp", bufs=2) as sqp,
            tc.tile_pool(name="ptp", bufs=1) as ptp,
            tc.tile_pool(name="srow", bufs=1) as srow,
            tc.tile_pool(name="bcp", bufs=1) as bcp,
            tc.tile_pool(name="hp", bufs=1) as hp,
            tc.tile_pool(name="op", bufs=1) as op_pool,
            tc.tile_pool(name="mmps", bufs=4, space="PSUM") as mmps,
            tc.tile_pool(name="stps", bufs=1, space="PSUM") as stps,
            tc.tile_pool(name="bcps", bufs=1, space="PSUM") as bcps,
            tc.tile_pool(name="ops", bufs=1, space="PSUM") as ops,
        ):
            # ---- resident constants / weights ----
            ones_col = const.tile([128, 1], BF16, tag="ones_col")
            nc.sync.dma_start(ones_col, onec_d[:, :])
            ones_row = const.tile([1, 128], F32R, tag="ones_row")
            nc.sync.dma_start(ones_row, oner_d[:, :])
            eps_sb = const.tile([1, 1], F32, tag="eps")
            nc.vector.memset(eps_sb, LN_EPS)
            b_sb = []
            for i in range(3):
                t = const.tile([128, JT], F32, tag=f"b{i}")
                nc.sync.dma_start(t, bsw_d[i, :, :])
                b_sb.append(t)
            f1b_sb = const.tile([128, JT], F32, tag="f1b")
            nc.sync.dma_start(f1b_sb, f1b_d[:, :])

            # Launch-head DMAs, two HW queues (SP + Act), ordered by first use:
            # attn weights + chunk-0 activations first, fc1 weights behind.
            def xtile(mod, dram, c, eng):
                t = xp.tile([128, KT, bc], BF16, tag=f"x{mod}")
                eng.dma_start(t, dram[c, :, :, :])
                return t

            # weight tiles split in j-halves so chunk-0 compute can start as
            # soon as the first half lands; head DMAs spread over THREE
            # queues (SP + Act HWDGE, Pool SWDGE) ordered by first use
            JH = JT // 2
            w_sb = [
                [
                    wres.tile([128, JH, KT, 128], BF16, tag=f"w{i}{h}", name=f"w{i}{h}")
                    for h in range(2)
                ]
                for i in range(3)
            ]
            g_sb = [
                [
                    wres.tile([128, JH, KT, 128], BF16, tag=f"g{i}{h}", name=f"g{i}{h}")
                    for h in range(2)
                ]
                for i in range(3)
            ]
            x0 = {}
            x0["a"] = xtile("a", xa_d, 0, nc.scalar)
            nc.sync.dma_start(w_sb[0][0], wtt_d[0, :, 0:JH, :, :])
            nc.scalar.dma_start(w_sb[0][1], wtt_d[0, :, JH:JT, :, :])
            x0["t"] = xtile("t", xt_d, 0, nc.scalar)
            x0["v"] = xtile("v", xv_d, 0, nc.sync)
            nc.sync.dma_start(w_sb[1][0], wtt_d[1, :, 0:JH, :, :])
            nc.scalar.dma_start(w_sb[1][1], wtt_d[1, :, JH:JT, :, :])
            nc.sync.dma_start(w_sb[2][0], wtt_d[2, :, 0:JH, :, :])
            nc.scalar.dma_start(w_sb[2][1], wtt_d[2, :, JH:JT, :, :])
            nc.sync.dma_start(g_sb[0][0], gtt_d[0, :, 0:JH, :, :])
            nc.scalar.dma_start(g_sb[1][0], gtt_d[1, :, 0:JH, :, :])
            nc.sync.dma_start(g_sb[2][0], gtt_d[2, :, 0:JH, :, :])
            nc.scalar.dma_start(g_sb[0][1], gtt_d[0, :, JH:JT, :, :])
            nc.sync.dma_start(g_sb[1][1], gtt_d[1, :, JH:JT, :, :])
            nc.scalar.dma_start(g_sb[2][1], gtt_d[2, :, JH:JT, :, :])
            f2hl_sb = const.tile([128, KT, 2 * OUT], BF16, tag="f2hl")
            nc.sync.dma_start(f2hl_sb, f2hl_d[:, :, :])

            for rep in range(reps):
              for c in range(nch):
                if c == 0 and rep == 0:
                    x_sb = x0
                else:
                    # all on the idle SP queue: issuing from Act would delay
                    # the transfer behind the previous chunk's Act compute
                    x_sb = {
                        "a": xtile("a", xa_d, c, nc.sync),
                        "t": xtile("t", xt_d, c, nc.sync),
                        "v": xtile("v", xv_d, c, nc.sync),
                    }

                u_sb = [None, None, None]
                stat = [None, None, None]  # (pu, py) f32 partials
                rowv = [None, None, None]  # (m_sb, r_sb)
                bcst = [None, None, None]  # (mbc, rbc) f32 sbuf

                def u_mm(i, jlo, jhi):
                    """attention matmuls for j in [jlo,jhi); evac+square+tree
                    ride along on DVE/Act (all-bf16 SBUF -> DVE 4x mode)."""
                    qm, km = blocks[i]
                    xq, xkv = x_sb[qm], x_sb[km]
                    if jlo == 0:
                        u_sb[i] = up.tile([128, JT, bc], BF16, tag="u", name="u")
                        stat[i] = (
                            ptp.tile([128, bc], BF16, tag="pu", name="pu"),
                            ptp.tile([128, bc], BF16, tag="py", name="py"),
                        )
                    u, (pu, py) = u_sb[i], stat[i]
                    for j in range(jlo, jhi):
                        u_ps = mmps.tile([128, bc], F32, tag="mm")
                        wt = w_sb[i][j // 4]
                        for k in range(KT):
                            nc.tensor.matmul(
                                u_ps, wt[:, j % 4, k, :], xkv[:, k, :],
                                start=(k == 0), stop=(k == KT - 1),
                            )
                        # evacuate PSUM + bias + residual in one DVE pass
                        nc.vector.scalar_tensor_tensor(
                            out=u[:, j, :], in0=u_ps,
                            scalar=b_sb[i][:, j : j + 1], in1=xq[:, j, :],
                            op0=ADD, op1=ADD,
                        )
                        sq = sqp.tile([128, bc], BF16, tag="sq")
                        nc.scalar.activation(sq, u[:, j, :], ACT_F.Square)
                        if j == 0:
                            nc.vector.tensor_copy(pu, u[:, j, :])
                            nc.vector.tensor_copy(py, sq)
                        else:
                            nc.vector.tensor_add(pu, pu, u[:, j, :])
                            nc.vector.tensor_add(py, py, sq)

                def s_mm(i):
                    """partition-reduce partials -> s1/s2, then small-op chain
                    to mean + inv-std rows (f32)."""
                    pu, py = stat[i]
                    # s1/s2 share one PSUM bank (base partitions 0 and 32)
                    st = stps.tile([64, bc], F32, tag="s12")
                    s1 = st[0:1, :]
                    s2 = st[32:33, :]
                    nc.tensor.matmul(s1, ones_col, pu, start=True, stop=True)
                    nc.tensor.matmul(s2, ones_col, py, start=True, stop=True)
                    # m/r rows in bf16: the broadcast copies round to bf16
                    # anyway, and bf16 stationaries avoid the slow f32r
                    # weight-load path in the broadcast matmuls
                    m_sb = srow.tile([1, bc], F32R, tag="m")
                    nc.scalar.mul(m_sb, s1, 1.0 / D)
                    msq = srow.tile([1, bc], F32, tag="msq")
                    nc.vector.tensor_mul(msq, m_sb, m_sb)
                    var = srow.tile([1, bc], F32, tag="var")
                    nc.vector.scalar_tensor_tensor(
                        out=var, in0=s2, scalar=1.0 / D, in1=msq, op0=MUL, op1=SUB
                    )
                    std = srow.tile([1, bc], F32, tag="std")
                    nc.scalar.activation(std, var, ACT_F.Sqrt, bias=eps_sb, scale=1.0)
                    r_sb = srow.tile([1, bc], F32R, tag="r")
                    with nc.allow_low_precision(reason="bf16 LN scale by design"):
                        nc.vector.reciprocal(r_sb, std)
                    rowv[i] = (m_sb, r_sb)

                def bc_mm(i):
                    """broadcast mean + inv-std across partitions (K=1 matmul),
                    copy to SBUF f32, then normalize u in place -> bf16 y."""
                    m_sb, r_sb = rowv[i]
                    mbc_ps = bcps.tile([128, bc], F32, tag="mbc")
                    nc.tensor.matmul(mbc_ps, ones_row, m_sb, start=True, stop=True)
                    rbc_ps = bcps.tile([128, bc], F32, tag="rbc")
                    nc.tensor.matmul(rbc_ps, ones_row, r_sb, start=True, stop=True)
                    mbc = bcp.tile([128, bc], BF16, tag="mbc_sb")
                    nc.vector.tensor_copy(mbc, mbc_ps)
                    rbc = bcp.tile([128, bc], BF16, tag="rbc_sb")
                    nc.vector.tensor_copy(rbc, rbc_ps)
                    bcst[i] = (mbc, rbc)

                def norm(i):
                    u = u_sb[i]
                    mbc, rbc = bcst[i]
                    for j in range(JT):
                        nc.vector.tensor_sub(u[:, j, :], u[:, j, :], mbc)
                        nc.vector.tensor_mul(u[:, j, :], u[:, j, :], rbc)

                def fc1(j2, ilo, ihi, z_ps):
                    for i in range(ilo, ihi):
                        gt = g_sb[i][j2 // 4]
                        for k in range(KT):
                            nc.tensor.matmul(
                                z_ps, gt[:, j2 % 4, k, :], u_sb[i][:, k, :],
                                start=(i == 0 and k == 0),
                                stop=(i == 2 and k == KT - 1),
                            )

                # ---- PE emission order: keep the PE streaming while the
                # stats latency chains (DVE/Act/Pool) resolve in parallel ----
                def relu_evac(j2, z_ps):
                    # on DVE (not Act): h = max(z + b, 0); keeps the fc1 PSUM
                    # recycle path off the Act queue (busy with squares)
                    nc.vector.tensor_scalar(
                        out=h_sb[:, j2, :], in0=z_ps,
                        scalar1=f1b_sb[:, j2 : j2 + 1], scalar2=0.0,
                        op0=ADD, op1=mybir.AluOpType.max,
                    )

                u_mm(0, 0, JT)
                u_mm(1, 0, 2)
                s_mm(0)
                u_mm(1, 2, JT)
                bc_mm(0)
                u_mm(2, 0, 2)
                s_mm(1)
                u_mm(2, 2, JT)
                norm(0)
                bc_mm(1)

                h_sb = hp.tile([128, JT, bc], BF16, tag="h")
                z01 = [
                    mmps.tile([128, bc], F32, tag="mm", name=f"z{t_}")
                    for t_ in range(2)
                ]
                fc1(0, 0, 1, z01[0])
                s_mm(2)
                norm(1)
                fc1(1, 0, 1, z01[1])
                fc1(0, 1, 2, z01[0])
                bc_mm(2)
                norm(2)
                fc1(1, 1, 2, z01[1])
                fc1(0, 2, 3, z01[0])
                relu_evac(0, z01[0])
                fc1(1, 2, 3, z01[1])
                relu_evac(1, z01[1])
                for j2 in range(2, JT):
                    z_ps = mmps.tile([128, bc], F32, tag="mm")
                    fc1(j2, 0, 3, z_ps)
                    relu_evac(j2, z_ps)

                # fc2 hi+lo split-bf16 folded into ONE 8-matmul pass: lhsT
                # [128, 6] = [hi | lo] per k-tile, output rows 0-2 + 3-5 both
                # hold partial sums; the final Act evac adds them. Full fp32
                # fc2-weight precision at single-pass PE cost.
                o_ps = ops.tile([2 * OUT, bc], F32, tag="o")
                for k in range(KT):
                    nc.tensor.matmul(
                        o_ps, f2hl_sb[:, k, :], h_sb[:, k, :],
                        start=(k == 0), stop=(k == KT - 1),
                    )
                o_sb = op_pool.tile([2 * OUT, bc], F32, tag="osb")
                nc.scalar.activation(o_sb, o_ps, ACT_F.Identity)
                nc.scalar.dma_start(out_d[c, :, :], o_sb)

    nc.compile()
    return nc


def _swizzle_weight(wt: np.ndarray, bf16) -> np.ndarray:
    """[D_in, D_out] contraction-major matrix -> [128, JT, KT, 128] where
    tile[p, j, k, jc] = wt[k*128+p, j*128+jc]."""
    kt, jt = wt.shape[0] // 128, wt.shape[1] // 128
    return np.ascontiguousarray(
        wt.reshape(kt, 128, jt, 128).transpose(1, 2, 0, 3).astype(bf16)
    )


def _swizzle_x(shard: np.ndarray, bc: int, bf16) -> np.ndarray:
    """[b_loc, D] activation shard -> chunked feature-major [nch, 128, KT, bc]."""
    b_loc = shard.shape[0]
    nch = b_loc // bc
    return np.ascontiguousarray(
        shard.astype(bf16).reshape(nch, bc, KT, 128).transpose(0, 3, 2, 1)
    )


def _prep_shared(w_qkv, b_qkv, w_o, b_o, ln_g, ln_b, fc1_w, fc1_b, fc2_w, fc2_b):
    import ml_dtypes

    bf16 = ml_dtypes.bfloat16
    f6 = np.float64
    wtt = np.empty((3, 128, JT, KT, 128), bf16)
    bsw = np.empty((3, 128, JT), np.float32)
    gtt = np.empty((3, 128, JT, KT, 128), bf16)
    f1b_full = fc1_b.astype(f6).copy()
    for i in range(3):
        wv, bv = w_qkv[i, 2].astype(f6), b_qkv[i, 2].astype(f6)
        wo, bo = w_o[i].astype(f6), b_o[i].astype(f6)
        w_i = wo @ wv                      # [j_out, d_in]
        bias_i = wo @ bv + bo              # [j_out]
        wtt[i] = _swizzle_weight(np.ascontiguousarray(w_i.T).astype(np.float32), bf16)
        bsw[i] = bias_i.astype(np.float32).reshape(JT, 128).T
        f_i = fc1_w[:, i * D : (i + 1) * D].astype(f6)   # [h, j]
        g_i = f_i * ln_g[i].astype(f6)[None, :]
        f1b_full += f_i @ ln_b[i].astype(f6)
        gtt[i] = _swizzle_weight(np.ascontiguousarray(g_i.T).astype(np.float32), bf16)
    f1bsw = f1b_full.astype(np.float32).reshape(JT, 128).T.copy()
    f2t = np.ascontiguousarray(fc2_w.astype(np.float32).T)     # [D, OUT]
    f2hi = f2t.astype(bf16)
    f2lo = (f2t - f2hi.astype(np.float32)).astype(bf16)
    f2hsw = f2hi.reshape(KT, 128, OUT).transpose(1, 0, 2)
    f2lsw = f2lo.reshape(KT, 128, OUT).transpose(1, 0, 2)
    # hi|lo concatenated into one [128, KT, 6] stationary: the 6-row matmul
    # output carries both partials; they are summed on the host
    f2hl = np.ascontiguousarray(np.concatenate([f2hsw, f2lsw], axis=2))
    return dict(
        wtt=wtt, gtt=gtt, bsw=bsw, f1b=np.ascontiguousarray(f1bsw),
        f2hl=f2hl,
        onec=np.ones((128, 1), bf16), oner=np.ones((1, 128), np.float32),
    )


def _make_in_maps(
    text_x, audio_x, video_x, w_qkv, b_qkv, w_o, b_o, ln_g, ln_b,
    fc1_w, fc1_b, fc2_w, fc2_b, bc,
):
    import ml_dtypes

    bf16 = ml_dtypes.bfloat16
    b_total = text_x.shape[0]
    b_loc = b_total // NCORES
    shared = _prep_shared(
        np.asarray(w_qkv), np.asarray(b_qkv), np.asarray(w_o), np.asarray(b_o),
        np.asarray(ln_g), np.asarray(ln_b), np.asarray(fc1_w),
        np.asarray(fc1_b), np.asarray(fc2_w), np.asarray(fc2_b),
    )
    in_maps = []
    for cidx in range(NCORES):
        sl = slice(cidx * b_loc, (cidx + 1) * b_loc)
        in_maps.append(
            dict(
                xt=_swizzle_x(text_x[sl], bc, bf16),
                xa=_swizzle_x(audio_x[sl], bc, bf16),
                xv=_swizzle_x(video_x[sl], bc, bf16),
                **shared,
            )
        )
    return in_maps


def kernel(
    text_x, audio_x, video_x, w_qkv, b_qkv, w_o, b_o, ln_g, ln_b,
    fc1_w, fc1_b, fc2_w, fc2_b, num_heads=16,
):
    text_x = np.asarray(text_x, np.float32)
    audio_x = np.asarray(audio_x, np.float32)
    video_x = np.asarray(video_x, np.float32)
    b_total = text_x.shape[0]
    b_loc = b_total // NCORES
    bc = min(512, b_loc)

    key = (b_loc, bc)
    if key not in _cache:
        _cache[key] = _build(b_loc, bc)
    nc = _cache[key]

    in_maps = _make_in_maps(
        text_x, audio_x, video_x, w_qkv, b_qkv, w_o, b_o, ln_g, ln_b,
        fc1_w, fc1_b, fc2_w, fc2_b, bc,
    )
    res = run_bass_kernel_spmd(nc, in_maps, core_ids=list(range(NCORES)))
    out = np.empty((b_total, OUT), np.float32)
    f2bv = np.asarray(fc2_b, np.float32).reshape(1, OUT)
    for cidx in range(NCORES):
        o = res.results[cidx]["outT"]  # [nch, 2*OUT, bc]: rows 0-2 hi, 3-5 lo
        o = o[:, :OUT, :] + o[:, OUT:, :]
        out[cidx * b_loc : (cidx + 1) * b_loc] = (
            o.transpose(0, 2, 1).reshape(b_loc, OUT) + f2bv
        )
    return out
